# revision 13
# baseline (speedup 1.0000x reference)
"""Trainium2 Bass kernel for nn_HSL1Loss (per-(batch,label) segment MSE loss).

loss = (1/B) * sum_b sum_{l=1..63, cnt>0} mean((feat[b][gt[b]==l] - l)^2)

Strategy: batch-data-parallel over 8 NeuronCores (2 images each). The wall
clock of a cached call is dominated by host->device transfer over the axon
tunnel (~75 MB/s), so the host packs both inputs into ONE uint8 tensor per
core: featmap as fp8 e4m3 followed by gt as uint8 (labels are 0..63) --
2 bytes/pixel = 32 MB total vs 128 MB for f32+int32 (loss rel err ~1e-4,
tolerance is 2e-2). Casting DMAs (gpsimd software DGE) widen fp8/u8 to bf16
during the load. On device, each [128, N] tile is reduced into per-(batch,label)
sum/count accumulators with 64 fused mask-multiply-accumulate passes
(scalar_tensor_tensor with accum_out, bf16) plus 64 count passes
(tensor_scalar is_equal with accum_out) on the Vector engine. Squared error
is produced on the Scalar engine. Partition reduce via ones-matmul on the
Tensor engine, division + final reduction on-device; host sums the 8
per-core partials (the scalar all-reduce).
"""
import numpy as np

import concourse.bass as bass
import concourse.bass_isa as bass_isa
import concourse.mybir as mybir
import concourse.tile as tile
from concourse.bass_utils import run_bass_kernel_spmd

# --- inline tile drain patch (kernel.py must be self-contained) -------------
from concourse import tile as _tile_mod


def _apply_drain_patch(max_waits=1):
    if getattr(_tile_mod.TileContext, "_drain_split_patched", False):
        return

    def _drain_and_barrier(self, tick_clock, wait_clock):
        drain_inst = self.nc.sync.drain()
        wait_clock.add_sem_waits(
            drain_inst.ins, _tile_mod.ScopedClock({None: tick_clock.global_clock})
        )
        si = drain_inst.ins.sync_info
        waits = list(si.on_wait or []) if si is not None else []
        if len(waits) > max_waits:
            upd = list(si.on_update or [])
            drain_inst.ins.sync_info = mybir.SyncInfo(
                on_wait=waits[:max_waits], on_update=upd
            )
            for i in range(max_waits, len(waits), max_waits):
                d2 = self.nc.sync.drain()
                d2.ins.sync_info = mybir.SyncInfo(
                    on_wait=waits[i : i + max_waits], on_update=[]
                )
        self.nc.all_engine_barrier()
        assert self.sems is not None
        popped = self.nc._tile_sem_poison_stack.pop()
        assert popped is self._sem_poison
        self.nc.clear_and_free_semaphores(list(self.sems.allocated().values()))
        self.nc.all_engine_barrier()

    _tile_mod.TileContext._drain_and_barrier = _drain_and_barrier
    _tile_mod.TileContext._drain_split_patched = True


_apply_drain_patch()

_MAX_INST_WAITS = 1
_wsplit_counter = [0]


def _split_waits(nc, k=_MAX_INST_WAITS):
    """Walrus in this toolchain rejects instructions with >k sem waits.
    Move excess waits onto same-engine NoOps inserted just before."""
    for fn in nc.m.functions:
        for bb in fn.blocks:
            il = list(bb.instructions)
            out = []
            changed = False
            for ins in il:
                si = ins.sync_info
                waits = list(si.on_wait or []) if si is not None else []
                if len(waits) > k:
                    changed = True
                    chunks = [waits[i : i + k] for i in range(0, len(waits), k)]
                    for ch in chunks[:-1]:
                        _wsplit_counter[0] += 1
                        nop = mybir.InstNoOp(
                            name=f"WSPLIT-{_wsplit_counter[0]}", ins=[], outs=[]
                        )
                        nop.engine = ins.engine
                        nop.sync_info = mybir.SyncInfo(on_wait=ch, on_update=[])
                        out.append(nop)
                    ins.sync_info = mybir.SyncInfo(
                        on_wait=chunks[-1], on_update=list(si.on_update or [])
                    )
                out.append(ins)
            if changed:
                bb.instructions = out

# --- problem constants (hardcoded per spec) ---------------------------------
B, H, W = 16, 1024, 1024
NUM_LABELS = 64
N_CORES = 8
BPC = B // N_CORES            # batches per core = 2
PX = H * W                    # pixels per batch = 1048576
P = 128
COLS = PX // P                # 8192 free-dim columns per batch
TILE_N = 4096
TPB = COLS // TILE_N          # tiles per batch = 2
NTILES = BPC * TPB            # tiles per core = 4
ROWB = 2 * COLS               # packed bytes per (batch, partition) row

F32 = mybir.dt.float32
U8 = mybir.dt.uint8
F8 = mybir.dt.float8e4
BF16 = mybir.dt.bfloat16
ALU = mybir.AluOpType

_CACHED_NC = None


def build_nc():
    global _CACHED_NC
    if _CACHED_NC is not None:
        return _CACHED_NC
    nc = bass.Bass()
    # packed input: per (batch, partition) row, bytes [0 : COLS) hold the
    # fp8 e4m3 featmap row; bytes [COLS : 2*COLS) hold the uint8 gt row.
    fgt = nc.dram_tensor("fgt", [BPC, P, ROWB], U8, kind="ExternalInput")
    out = nc.dram_tensor("out", [1, 1], F32, kind="ExternalOutput")

    with tile.TileContext(nc) as tc:
        with (
            tc.tile_pool(name="fin", bufs=2) as fin_pool,
            tc.tile_pool(name="gbf", bufs=2) as gbf_pool,
            tc.tile_pool(name="sq", bufs=2) as sq_pool,
            tc.tile_pool(name="dbf", bufs=2) as d_pool,
            tc.tile_pool(name="dum", bufs=1) as dum_pool,
            tc.tile_pool(name="acc", bufs=1) as acc_pool,
            tc.tile_pool(name="fini", bufs=1) as fini_pool,
        ):
            # per-(label, tile) accumulator columns: col = l*NTILES + t
            acc_s = acc_pool.tile([P, NUM_LABELS * NTILES], F32)
            acc_c = acc_pool.tile([P, NUM_LABELS * NTILES], F32)
            vdum = [dum_pool.tile([P, TILE_N], BF16, name=f"vd{i}", tag=f"vd{i}") for i in range(4)]

            for t in range(NTILES):
                b, tb = divmod(t, TPB)
                # casting DMAs (software DGE): fp8 -> bf16 and u8 -> bf16
                f_t = fin_pool.tile([P, TILE_N], BF16)
                nc.gpsimd.dma_start(
                    out=f_t[:],
                    in_=fgt[b, :, TILE_N * tb : TILE_N * (tb + 1)].bitcast(F8),
                )
                g_bf = gbf_pool.tile([P, TILE_N], BF16)
                nc.gpsimd.dma_start(
                    out=g_bf[:],
                    in_=fgt[b, :, COLS + TILE_N * tb : COLS + TILE_N * (tb + 1)],
                )

                d_bf = d_pool.tile([P, TILE_N], BF16)
                nc.vector.tensor_tensor(
                    out=d_bf[:], in0=f_t[:], in1=g_bf[:], op=ALU.subtract
                )
                sq = sq_pool.tile([P, TILE_N], BF16)
                nc.scalar.activation(
                    sq[:], d_bf[:], mybir.ActivationFunctionType.Square
                )

                for l in range(NUM_LABELS):
                    col = l * NTILES + t
                    nc.vector.scalar_tensor_tensor(
                        out=vdum[l % 4][:],
                        in0=g_bf[:],
                        scalar=float(l),
                        in1=sq[:],
                        op0=ALU.is_equal,
                        op1=ALU.mult,
                        accum_out=acc_s[:, col : col + 1],
                    )
                for l in range(NUM_LABELS):
                    col = l * NTILES + t
                    nc.vector.tensor_scalar(
                        out=vdum[(l + 2) % 4][:],
                        in0=g_bf[:],
                        scalar1=float(l),
                        scalar2=0.0,
                        op0=ALU.is_equal,
                        op1=ALU.add,
                        accum_out=acc_c[:, col : col + 1],
                    )

            # ---- final reduction (tiny) ----
            # X-reduce tiles-per-batch: [128, l, BPC, TPB] -> [128, l*BPC]
            red_s = fini_pool.tile([P, NUM_LABELS * BPC], F32)
            red_c = fini_pool.tile([P, NUM_LABELS * BPC], F32)
            nc.vector.tensor_reduce(
                out=red_s[:],
                in_=acc_s[:].rearrange("p (l b t) -> p (l b) t", l=NUM_LABELS, b=BPC),
                axis=mybir.AxisListType.X,
                op=ALU.add,
            )
            nc.vector.tensor_reduce(
                out=red_c[:],
                in_=acc_c[:].rearrange("p (l b t) -> p (l b) t", l=NUM_LABELS, b=BPC),
                axis=mybir.AxisListType.X,
                op=ALU.add,
            )
            # partition reduce via ones-matmul on the Tensor engine
            nl = NUM_LABELS * BPC
            ones = fini_pool.tile([P, 1], F32)
            nc.vector.memset(ones[:], 1.0)
            with tc.tile_pool(name="ps", bufs=1, space="PSUM") as psum_pool:
                ps_s = psum_pool.tile([1, nl], F32)
                ps_c = psum_pool.tile([1, nl], F32)
                nc.tensor.matmul(ps_s[:], lhsT=ones[:], rhs=red_s[:], start=True, stop=True)
                nc.tensor.matmul(ps_c[:], lhsT=ones[:], rhs=red_c[:], start=True, stop=True)
                par_s = fini_pool.tile([1, nl], F32)
                par_c = fini_pool.tile([1, nl], F32)
                nc.vector.tensor_copy(par_s[:], ps_s[:])
                nc.vector.tensor_copy(par_c[:], ps_c[:])
            # scalar math on partition-0 row: [1, nl] with col = l*BPC + b
            cclamp = fini_pool.tile([1, nl], F32)
            nc.vector.tensor_scalar(
                out=cclamp[:], in0=par_c[:, :], scalar1=1.0, scalar2=None, op0=ALU.max
            )
            inv = fini_pool.tile([1, nl], F32)
            nc.vector.reciprocal(inv[:], cclamp[:])
            contrib = fini_pool.tile([1, nl], F32)
            nc.vector.tensor_tensor(
                out=contrib[:], in0=par_s[:, :], in1=inv[:], op=ALU.mult
            )
            mask = fini_pool.tile([1, nl], F32)
            nc.vector.tensor_scalar(
                out=mask[:], in0=par_c[:, :], scalar1=0.5, scalar2=None, op0=ALU.is_ge
            )
            gated = fini_pool.tile([1, nl], F32)
            nc.vector.tensor_tensor(
                out=gated[:], in0=contrib[:], in1=mask[:], op=ALU.mult
            )
            # sum over labels 1..63, both batches: cols [BPC:] skip label 0
            loss = fini_pool.tile([1, 1], F32)
            nc.vector.tensor_reduce(
                out=loss[:],
                in_=gated[:, BPC:],
                axis=mybir.AxisListType.X,
                op=ALU.add,
            )
            nc.gpsimd.dma_start(out=out[:, :], in_=loss[:])
    _split_waits(nc)
    _CACHED_NC = nc
    return nc


_F8LUT = None


def _f8_lut() -> np.ndarray:
    """uint8 LUT: IEEE f16 bit pattern -> fp8 e4m3 byte (the ml_dtypes
    float8_e4m3 encoding, which is what mybir.dt.float8e4 maps to)."""
    global _F8LUT
    if _F8LUT is None:
        import ml_dtypes

        all_f16 = np.arange(65536, dtype=np.uint16).view(np.float16)
        with np.errstate(invalid="ignore", over="ignore"):
            _F8LUT = (
                all_f16.astype(np.float32).astype(ml_dtypes.float8_e4m3).view(np.uint8)
            )
    return _F8LUT


def _pack_inputs(featmap: np.ndarray, gt: np.ndarray) -> np.ndarray:
    """Pack featmap (as fp8 e4m3 bytes, via f16 + LUT) + gt (as uint8) into
    one [B, P, 2*COLS] uint8 array."""
    f = np.ascontiguousarray(featmap, dtype=np.float32).reshape(B, PX)
    g = np.asarray(gt).reshape(B, P, COLS)
    lut = _f8_lut()
    buf = np.empty((B, P, ROWB), np.uint8)
    fh = f.astype(np.float16)
    buf[:, :, :COLS] = lut[fh.view(np.uint16)].reshape(B, P, COLS)
    buf[:, :, COLS:] = g  # int32 -> uint8 (values 0..63)
    return buf


def kernel(featmap: np.ndarray, gt: np.ndarray) -> np.ndarray:
    assert featmap.shape == (B, 1, H, W) and gt.shape == (B, 1, H, W)
    buf = _pack_inputs(featmap, gt)
    nc = build_nc()
    in_maps = [{"fgt": buf[c * BPC : (c + 1) * BPC]} for c in range(N_CORES)]
    res = run_bass_kernel_spmd(nc, in_maps, core_ids=list(range(N_CORES)))
    total = sum(float(r["out"][0, 0]) for r in res.results)
    return np.float32(total / B)


# revision 15
# speedup vs baseline: 1.6637x; 1.6637x over previous
"""Trainium2 Bass kernel for nn_HSL1Loss (per-(batch,label) segment MSE loss).

loss = (1/B) * sum_b sum_{l=1..63, cnt>0} mean((feat[b][gt[b]==l] - l)^2)

Strategy: batch-data-parallel over 8 NeuronCores (2 images each). The wall
clock of a cached call is dominated by host->device transfer over the axon
tunnel (~75 MB/s), so the host packs both inputs into ONE uint8 tensor per
core: featmap as fp8 e4m3 followed by gt as uint8 (labels are 0..63) --
2 bytes/pixel = 32 MB total vs 128 MB for f32+int32 (loss rel err ~1e-4,
tolerance is 2e-2). Casting DMAs (gpsimd software DGE) widen fp8/u8 to bf16
during the load. On device, each [128, N] tile is reduced into per-(batch,label)
sum/count accumulators with 64 fused mask-multiply-accumulate passes
(scalar_tensor_tensor with accum_out, bf16) plus 64 count passes
(tensor_scalar is_equal with accum_out) on the Vector engine. Squared error
is produced on the Scalar engine. Partition reduce via ones-matmul on the
Tensor engine, division + final reduction on-device; host sums the 8
per-core partials (the scalar all-reduce).
"""
import numpy as np

import concourse.bass as bass
import concourse.bass_isa as bass_isa
import concourse.mybir as mybir
import concourse.tile as tile
from concourse.bass_utils import run_bass_kernel_spmd

# --- inline tile drain patch (kernel.py must be self-contained) -------------
from concourse import tile as _tile_mod


def _apply_drain_patch(max_waits=1):
    if getattr(_tile_mod.TileContext, "_drain_split_patched", False):
        return

    def _drain_and_barrier(self, tick_clock, wait_clock):
        drain_inst = self.nc.sync.drain()
        wait_clock.add_sem_waits(
            drain_inst.ins, _tile_mod.ScopedClock({None: tick_clock.global_clock})
        )
        si = drain_inst.ins.sync_info
        waits = list(si.on_wait or []) if si is not None else []
        if len(waits) > max_waits:
            upd = list(si.on_update or [])
            drain_inst.ins.sync_info = mybir.SyncInfo(
                on_wait=waits[:max_waits], on_update=upd
            )
            for i in range(max_waits, len(waits), max_waits):
                d2 = self.nc.sync.drain()
                d2.ins.sync_info = mybir.SyncInfo(
                    on_wait=waits[i : i + max_waits], on_update=[]
                )
        self.nc.all_engine_barrier()
        assert self.sems is not None
        popped = self.nc._tile_sem_poison_stack.pop()
        assert popped is self._sem_poison
        self.nc.clear_and_free_semaphores(list(self.sems.allocated().values()))
        self.nc.all_engine_barrier()

    _tile_mod.TileContext._drain_and_barrier = _drain_and_barrier
    _tile_mod.TileContext._drain_split_patched = True


_apply_drain_patch()

_MAX_INST_WAITS = 1
_wsplit_counter = [0]


def _split_waits(nc, k=_MAX_INST_WAITS):
    """Walrus in this toolchain rejects instructions with >k sem waits.
    Move excess waits onto same-engine NoOps inserted just before."""
    for fn in nc.m.functions:
        for bb in fn.blocks:
            il = list(bb.instructions)
            out = []
            changed = False
            for ins in il:
                si = ins.sync_info
                waits = list(si.on_wait or []) if si is not None else []
                if len(waits) > k:
                    changed = True
                    chunks = [waits[i : i + k] for i in range(0, len(waits), k)]
                    for ch in chunks[:-1]:
                        _wsplit_counter[0] += 1
                        nop = mybir.InstNoOp(
                            name=f"WSPLIT-{_wsplit_counter[0]}", ins=[], outs=[]
                        )
                        nop.engine = ins.engine
                        nop.sync_info = mybir.SyncInfo(on_wait=ch, on_update=[])
                        out.append(nop)
                    ins.sync_info = mybir.SyncInfo(
                        on_wait=chunks[-1], on_update=list(si.on_update or [])
                    )
                out.append(ins)
            if changed:
                bb.instructions = out

# --- problem constants (hardcoded per spec) ---------------------------------
B, H, W = 16, 1024, 1024
NUM_LABELS = 64
N_CORES = 8
BPC = B // N_CORES            # batches per core = 2
PX = H * W                    # pixels per batch = 1048576
P = 128
COLS = PX // P                # 8192 free-dim columns per batch
TILE_N = 4096
TPB = COLS // TILE_N          # tiles per batch = 2
NTILES = BPC * TPB            # tiles per core = 4
ROWB = 2 * COLS               # packed bytes per (batch, partition) row

F32 = mybir.dt.float32
U8 = mybir.dt.uint8
F8 = mybir.dt.float8e4
BF16 = mybir.dt.bfloat16
ALU = mybir.AluOpType

_CACHED_NC = None


def build_nc():
    global _CACHED_NC
    if _CACHED_NC is not None:
        return _CACHED_NC
    nc = bass.Bass()
    # packed input: per (batch, partition) row, bytes [0 : COLS) hold the
    # fp8 e4m3 featmap row; bytes [COLS : 2*COLS) hold the uint8 gt row.
    fgt = nc.dram_tensor("fgt", [BPC, P, ROWB], U8, kind="ExternalInput")
    out = nc.dram_tensor("out", [1, 1], F32, kind="ExternalOutput")

    with tile.TileContext(nc) as tc:
        with (
            tc.tile_pool(name="fin", bufs=2) as fin_pool,
            tc.tile_pool(name="gbf", bufs=2) as gbf_pool,
            tc.tile_pool(name="sq", bufs=2) as sq_pool,
            tc.tile_pool(name="dbf", bufs=2) as d_pool,
            tc.tile_pool(name="dum", bufs=1) as dum_pool,
            tc.tile_pool(name="acc", bufs=1) as acc_pool,
            tc.tile_pool(name="fini", bufs=1) as fini_pool,
        ):
            # per-(label, tile) accumulator columns: col = l*NTILES + t
            acc_s = acc_pool.tile([P, NUM_LABELS * NTILES], F32)
            acc_c = acc_pool.tile([P, NUM_LABELS * NTILES], F32)
            vdum = [dum_pool.tile([P, TILE_N], BF16, name=f"vd{i}", tag=f"vd{i}") for i in range(4)]

            for t in range(NTILES):
                b, tb = divmod(t, TPB)
                # casting DMAs (software DGE): fp8 -> bf16 and u8 -> bf16
                f_t = fin_pool.tile([P, TILE_N], BF16)
                nc.gpsimd.dma_start(
                    out=f_t[:],
                    in_=fgt[b, :, TILE_N * tb : TILE_N * (tb + 1)].bitcast(F8),
                )
                g_bf = gbf_pool.tile([P, TILE_N], BF16)
                nc.gpsimd.dma_start(
                    out=g_bf[:],
                    in_=fgt[b, :, COLS + TILE_N * tb : COLS + TILE_N * (tb + 1)],
                )

                d_bf = d_pool.tile([P, TILE_N], BF16)
                nc.vector.tensor_tensor(
                    out=d_bf[:], in0=f_t[:], in1=g_bf[:], op=ALU.subtract
                )
                sq = sq_pool.tile([P, TILE_N], BF16)
                nc.scalar.activation(
                    sq[:], d_bf[:], mybir.ActivationFunctionType.Square
                )

                for l in range(NUM_LABELS):
                    col = l * NTILES + t
                    nc.vector.scalar_tensor_tensor(
                        out=vdum[l % 4][:],
                        in0=g_bf[:],
                        scalar=float(l),
                        in1=sq[:],
                        op0=ALU.is_equal,
                        op1=ALU.mult,
                        accum_out=acc_s[:, col : col + 1],
                    )
                for l in range(NUM_LABELS):
                    col = l * NTILES + t
                    nc.vector.tensor_scalar(
                        out=vdum[(l + 2) % 4][:],
                        in0=g_bf[:],
                        scalar1=float(l),
                        scalar2=0.0,
                        op0=ALU.is_equal,
                        op1=ALU.add,
                        accum_out=acc_c[:, col : col + 1],
                    )

            # ---- final reduction (tiny) ----
            # X-reduce tiles-per-batch: [128, l, BPC, TPB] -> [128, l*BPC]
            red_s = fini_pool.tile([P, NUM_LABELS * BPC], F32)
            red_c = fini_pool.tile([P, NUM_LABELS * BPC], F32)
            nc.vector.tensor_reduce(
                out=red_s[:],
                in_=acc_s[:].rearrange("p (l b t) -> p (l b) t", l=NUM_LABELS, b=BPC),
                axis=mybir.AxisListType.X,
                op=ALU.add,
            )
            nc.vector.tensor_reduce(
                out=red_c[:],
                in_=acc_c[:].rearrange("p (l b t) -> p (l b) t", l=NUM_LABELS, b=BPC),
                axis=mybir.AxisListType.X,
                op=ALU.add,
            )
            # partition reduce via ones-matmul on the Tensor engine
            nl = NUM_LABELS * BPC
            ones = fini_pool.tile([P, 1], F32)
            nc.vector.memset(ones[:], 1.0)
            with tc.tile_pool(name="ps", bufs=1, space="PSUM") as psum_pool:
                ps_s = psum_pool.tile([1, nl], F32)
                ps_c = psum_pool.tile([1, nl], F32)
                nc.tensor.matmul(ps_s[:], lhsT=ones[:], rhs=red_s[:], start=True, stop=True)
                nc.tensor.matmul(ps_c[:], lhsT=ones[:], rhs=red_c[:], start=True, stop=True)
                par_s = fini_pool.tile([1, nl], F32)
                par_c = fini_pool.tile([1, nl], F32)
                nc.vector.tensor_copy(par_s[:], ps_s[:])
                nc.vector.tensor_copy(par_c[:], ps_c[:])
            # scalar math on partition-0 row: [1, nl] with col = l*BPC + b
            cclamp = fini_pool.tile([1, nl], F32)
            nc.vector.tensor_scalar(
                out=cclamp[:], in0=par_c[:, :], scalar1=1.0, scalar2=None, op0=ALU.max
            )
            inv = fini_pool.tile([1, nl], F32)
            nc.vector.reciprocal(inv[:], cclamp[:])
            contrib = fini_pool.tile([1, nl], F32)
            nc.vector.tensor_tensor(
                out=contrib[:], in0=par_s[:, :], in1=inv[:], op=ALU.mult
            )
            mask = fini_pool.tile([1, nl], F32)
            nc.vector.tensor_scalar(
                out=mask[:], in0=par_c[:, :], scalar1=0.5, scalar2=None, op0=ALU.is_ge
            )
            gated = fini_pool.tile([1, nl], F32)
            nc.vector.tensor_tensor(
                out=gated[:], in0=contrib[:], in1=mask[:], op=ALU.mult
            )
            # sum over labels 1..63, both batches: cols [BPC:] skip label 0
            loss = fini_pool.tile([1, 1], F32)
            nc.vector.tensor_reduce(
                out=loss[:],
                in_=gated[:, BPC:],
                axis=mybir.AxisListType.X,
                op=ALU.add,
            )
            nc.gpsimd.dma_start(out=out[:, :], in_=loss[:])
    _split_waits(nc)
    _CACHED_NC = nc
    return nc


_F8LUT = None


def _f8_lut() -> np.ndarray:
    """uint8 LUT: bf16 bit pattern (high u16 of an f32) -> fp8 e4m3 byte
    (the ml_dtypes float8_e4m3 encoding, which is what mybir.dt.float8e4
    maps to)."""
    global _F8LUT
    if _F8LUT is None:
        import ml_dtypes

        all_bf16 = np.arange(65536, dtype=np.uint16).view(ml_dtypes.bfloat16)
        with np.errstate(invalid="ignore", over="ignore"):
            _F8LUT = (
                all_bf16.astype(np.float32).astype(ml_dtypes.float8_e4m3).view(np.uint8)
            )
    return _F8LUT


def _pack_inputs(featmap: np.ndarray, gt: np.ndarray) -> np.ndarray:
    """Pack featmap (as fp8 e4m3 bytes, via bf16-truncation + LUT) + gt (as
    uint8) into one [B, P, 2*COLS] uint8 array."""
    f = np.ascontiguousarray(featmap, dtype=np.float32).reshape(B, PX)
    g = np.asarray(gt).reshape(B, P, COLS)
    lut = _f8_lut()
    buf = np.empty((B, P, ROWB), np.uint8)
    buf[:, :, :COLS] = lut[f.view(np.uint16)[:, 1::2]].reshape(B, P, COLS)
    buf[:, :, COLS:] = g  # int32 -> uint8 (values 0..63)
    return buf


_EXEC_CACHE = None


def _get_exec():
    """Build (once) a jitted shard_map program around the bass_exec custom
    call -- the same lowering run_bass_kernel_spmd uses under axon, but
    cached across kernel() calls so repeat calls skip retrace + BIR
    re-hashing (~0.4 s/call)."""
    global _EXEC_CACHE
    if _EXEC_CACHE is None:
        import jax
        from jax.sharding import Mesh, PartitionSpec
        from jax.experimental.shard_map import shard_map
        from concourse.bass2jax import (
            _bass_exec_p,
            install_neuronx_cc_hook,
            partition_id_tensor,
        )

        nc = build_nc()
        install_neuronx_cc_hook()
        partition_name = (
            nc.partition_id_tensor.name if nc.partition_id_tensor else None
        )
        in_names, out_names, out_avals = [], [], []
        for alloc in nc.m.functions[0].allocations:
            if not isinstance(alloc, mybir.MemoryLocationSet):
                continue
            name = alloc.memorylocations[0].name
            if alloc.kind == "ExternalInput":
                if name != partition_name:
                    in_names.append(name)
            elif alloc.kind == "ExternalOutput":
                out_names.append(name)
                out_avals.append(
                    jax.core.ShapedArray(
                        tuple(alloc.tensor_shape), mybir.dt.np(alloc.dtype)
                    )
                )
        assert in_names == ["fgt"] and out_names == ["out"]
        n_params, n_outs = len(in_names), len(out_avals)
        all_names = list(in_names) + out_names
        if partition_name is not None:
            all_names.append(partition_name)
        donate = tuple(range(n_params, n_params + n_outs))

        def _body(*args):
            operands = list(args)
            if partition_name is not None:
                operands.append(partition_id_tensor())
            outs = _bass_exec_p.bind(
                *operands,
                out_avals=tuple(out_avals),
                in_names=tuple(all_names),
                out_names=tuple(out_names),
                lowering_input_output_aliases=(),
                sim_require_finite=True,
                sim_require_nnan=True,
                nc=nc,
            )
            return tuple(outs)

        devices = jax.devices()[:N_CORES]
        mesh = Mesh(np.asarray(devices), ("core",))
        _EXEC_CACHE = jax.jit(
            shard_map(
                _body,
                mesh=mesh,
                in_specs=(PartitionSpec("core"),) * (n_params + n_outs),
                out_specs=(PartitionSpec("core"),) * n_outs,
                check_rep=False,
            ),
            donate_argnums=donate,
            keep_unused=True,
        )
    return _EXEC_CACHE


def kernel(featmap: np.ndarray, gt: np.ndarray) -> np.ndarray:
    assert featmap.shape == (B, 1, H, W) and gt.shape == (B, 1, H, W)
    buf = _pack_inputs(featmap, gt)
    try:
        sharded = _get_exec()
        out = sharded(buf, np.zeros((N_CORES, 1), np.float32))
        parts = np.asarray(out[0]).reshape(N_CORES)
        return np.float32(parts.sum(dtype=np.float64) / B)
    except Exception:
        # robust fallback: the library SPMD path (same NEFF, fresh jit)
        nc = build_nc()
        in_maps = [{"fgt": buf[c * BPC : (c + 1) * BPC]} for c in range(N_CORES)]
        res = run_bass_kernel_spmd(nc, in_maps, core_ids=list(range(N_CORES)))
        total = sum(float(r["out"][0, 0]) for r in res.results)
        return np.float32(total / B)
